# revision 15
# baseline (speedup 1.0000x reference)
"""Trainium2 Bass kernel for nn_BipartitePooling (GATv2 bipartite pooling).

Sharding: one graph per NeuronCore (8 graphs, 8 cores, no collectives).
Per core, for its graph's nodes (padded to N_CAP columns):

  h_srcT[f,n] = W_l^T @ xT                      (PE)
  logit[4r+h, n] = A^T lrelu(h_srcT + srT[:,r]) (split as 0.8*relu + 0.2*linear:
      relu stream on ACT/DVE, channel-reduce + linear correction + pad mask
      as PE matmuls accumulating in PSUM)
  p = exp(logit)                                (ACT; softmax max-shift skipped,
                                                 logits are O(+-10) so exp is safe)
  O[m,f], s[m] = p @ [h_src | 1]                (PE, via DMA-transposed p)
  out = O / s + bias                            (DVE)

Seed-side prep (srT = seed@W_r, block-diag att matrix, Q correction, pad
masks) is tiny weight preprocessing done on host in numpy.
"""
import os
import sys
from contextlib import ExitStack

import numpy as np

for _p in ("/root/.axon_site", "/root/.axon_site/_ro/trn_rl_repo",
           "/root/.axon_site/_ro/pypackages", "/opt/trn_rl_repo"):
    if os.path.isdir(_p) and _p not in sys.path:
        sys.path.append(_p)

import concourse.bass as bass
import concourse.bacc as bacc
import concourse.tile as tile
import concourse.mybir as mybir
from concourse.bass_utils import run_bass_kernel_spmd

F32 = mybir.dt.float32
BF16 = mybir.dt.bfloat16
AF = mybir.ActivationFunctionType
ALU = mybir.AluOpType
NP_BF16 = mybir.dt.np(BF16)

B, F, RATIO, H, C = 8, 128, 32, 4, 32
N_CAP = 2304
NT = N_CAP // 128                       # 18 node tiles
CHUNKS = [(o, min(512, N_CAP - o)) for o in range(0, N_CAP, 512)]
ACT_ROW_MOD = 3                         # r % 3 == 0 -> ACT relu, else DVE

_CACHE = {}


def build_bass(n_cap=N_CAP):
    nc = bacc.Bacc("TRN2", target_bir_lowering=False, debug=False)
    xT = nc.declare_dram_parameter("xT", [128, n_cap], BF16, isOutput=False)
    wl = nc.declare_dram_parameter("W_l", [128, 128], BF16, isOutput=False)
    srT = nc.declare_dram_parameter("srT", [128, 32], F32, isOutput=False)
    a8 = nc.declare_dram_parameter("A8", [128, 4], BF16, isOutput=False)
    abig = nc.declare_dram_parameter("A_big", [128, RATIO * 128], BF16,
                                     isOutput=False)
    selq = nc.declare_dram_parameter("SelQ", [6, 128], BF16, isOutput=False)
    mask = nc.declare_dram_parameter("maskbias", [2, n_cap], BF16, isOutput=False)
    bias1 = nc.declare_dram_parameter("bias1", [1, 128], F32, isOutput=False)
    outp = nc.declare_dram_parameter("out", [128, 128], F32, isOutput=True)

    chunks = [(o, min(512, n_cap - o)) for o in range(0, n_cap, 512)]
    nt = n_cap // 128

    with ExitStack() as ctx:
        tc = ctx.enter_context(tile.TileContext(nc))
        consts = ctx.enter_context(tc.tile_pool(name="consts", bufs=1))
        big = ctx.enter_context(tc.tile_pool(name="big", bufs=1))
        rpool = ctx.enter_context(tc.tile_pool(name="rpool", bufs=4))
        ps_tmp = ctx.enter_context(tc.tile_pool(name="ps_tmp", bufs=2, space="PSUM"))
        ps_lg = ctx.enter_context(tc.tile_pool(name="ps_lg", bufs=1, space="PSUM"))
        ps_os = ctx.enter_context(tc.tile_pool(name="ps_os", bufs=1, space="PSUM"))

        # ---- constants in ----
        wl_sb = consts.tile([128, 128], BF16, tag="wl")
        nc.sync.dma_start(out=wl_sb, in_=wl[:])
        a8_sb = consts.tile([128, 4], BF16, tag="a8")
        nc.sync.dma_start(out=a8_sb, in_=a8[:])
        selq_sb = consts.tile([6, 128], BF16, tag="selq")
        nc.sync.dma_start(out=selq_sb, in_=selq[:])
        srT_sb = consts.tile([128, 32], F32, tag="srt")
        nc.sync.dma_start(out=srT_sb, in_=srT[:])
        biasb_sb = consts.tile([128, 128], F32, tag="biasb")
        b1 = bias1[0:1, :]
        nc.gpsimd.dma_start(
            out=biasb_sb,
            in_=bass.AP(tensor=b1.tensor, offset=b1.offset, ap=[[0, 128], [1, 128]]),
        )
        abig_sb = consts.tile([128, RATIO * 128], BF16, tag="abig")
        for o in range(0, RATIO * 128, 1024):
            nc.sync.dma_start(out=abig_sb[:, o:o + 1024], in_=abig[:, o:o + 1024])

        # ---- x in (chunked so matmuls can start early) ----
        xT_sb = big.tile([128, n_cap], BF16, tag="xT")
        for off, sz in chunks:
            nc.sync.dma_start(out=xT_sb[:, off:off + sz], in_=xT[:, off:off + sz])

        # ---- h_srcT = W_l^T @ xT ----
        hsrcT_sb = big.tile([128, n_cap], BF16, tag="hsrcT")
        for ci, (off, sz) in enumerate(chunks):
            pt = ps_tmp.tile([128, 512], F32, tag="tmp")
            nc.tensor.matmul(pt[:, :sz], lhsT=wl_sb, rhs=xT_sb[:, off:off + sz],
                             start=True, stop=True)
            if ci % 2 == 0:
                nc.vector.tensor_copy(hsrcT_sb[:, off:off + sz], pt[:, :sz])
            else:
                nc.scalar.copy(hsrcT_sb[:, off:off + sz], pt[:, :sz])

        # ---- h_src tiles [n,f] (+ ones column per tile for the s-sum) ----
        hsrc_sb = big.tile([128, nt * 129], BF16, tag="hsrc")
        nc.vector.memset(hsrc_sb[:, 128::129], 1.0)
        for t in range(nt):
            pt = ps_tmp.tile([128, 512], F32, tag="tmp")
            nc.tensor.matmul(pt[:, :128], lhsT=xT_sb[:, 128 * t:128 * (t + 1)],
                             rhs=wl_sb, start=True, stop=True)
            if t % 2 == 0:
                nc.vector.tensor_copy(hsrc_sb[:, 129 * t:129 * t + 128], pt[:, :128])
            else:
                nc.scalar.copy(hsrc_sb[:, 129 * t:129 * t + 128], pt[:, :128])

        # ---- linear-term rhs: rows 0-3 = A8^T h_srcT, row 4 = 1, row 5 = mask ----
        rhsP_sb = big.tile([6, n_cap], BF16, tag="rhsP")
        nc.sync.dma_start(out=rhsP_sb[4:6, :], in_=mask[:])
        for ci, (off, sz) in enumerate(chunks):
            pt = ps_tmp.tile([128, 512], F32, tag="tmp")
            nc.tensor.matmul(pt[0:4, :sz], lhsT=a8_sb, rhs=hsrcT_sb[:, off:off + sz],
                             start=True, stop=True)
            nc.vector.tensor_copy(rhsP_sb[0:4, off:off + sz], pt[0:4, :sz])

        # ---- logit PSUM chunks: linear correction + mask first (start=True) ----
        lg = []
        for ci, (off, sz) in enumerate(chunks):
            g = ps_lg.tile([128, sz], F32, tag=f"lg{ci}")
            nc.tensor.matmul(g[:, :], lhsT=selq_sb, rhs=rhsP_sb[:, off:off + sz],
                             start=True, stop=False, skip_group_check=True)
            lg.append(g)

        # ---- relu stream + channel reduce, one seed r at a time ----
        for r in range(RATIO):
            R = rpool.tile([128, n_cap], BF16, tag="R")
            bcol = srT_sb[:, r:r + 1]
            if r % ACT_ROW_MOD == 0:
                nc.scalar.activation(R, hsrcT_sb, AF.Relu, bias=bcol, scale=1.0)
            else:
                nc.vector.tensor_scalar(R, hsrcT_sb, scalar1=bcol, scalar2=0.0,
                                        op0=ALU.add, op1=ALU.max)
            m_hi = 4 * (r + 1)
            for ci, (off, sz) in enumerate(chunks):
                nc.tensor.matmul(lg[ci][0:m_hi, :],
                                 lhsT=abig_sb[:, 128 * r:128 * r + m_hi],
                                 rhs=R[:, off:off + sz],
                                 start=False, stop=(r == RATIO - 1),
                                 skip_group_check=True)

        # ---- softmax numerator ----
        logit_sb = big.tile([128, n_cap], F32, tag="logit")
        for ci, (off, sz) in enumerate(chunks):
            if ci % 2 == 0:
                nc.vector.tensor_copy(logit_sb[:, off:off + sz], lg[ci][:, :])
            else:
                nc.scalar.copy(logit_sb[:, off:off + sz], lg[ci][:, :])
        p_sb = big.tile([128, n_cap], BF16, tag="p")
        nc.scalar.activation(p_sb, logit_sb, AF.Exp)

        # ---- transpose p for the aggregation contraction over n ----
        pT_sb = big.tile([128, nt * 128], BF16, tag="pT")
        for t in range(nt):
            nc.sync.dma_start_transpose(pT_sb[:, 128 * t:128 * (t + 1)],
                                        p_sb[:, 128 * t:128 * (t + 1)])

        # ---- O[m,f] and s[m] in one accumulation ----
        os_ps = ps_os.tile([128, 129], F32, tag="os")
        for t in range(nt):
            nc.tensor.matmul(os_ps[:, :], lhsT=pT_sb[:, 128 * t:128 * (t + 1)],
                             rhs=hsrc_sb[:, 129 * t:129 * (t + 1)],
                             start=(t == 0), stop=(t == nt - 1))

        srecip = big.tile([128, 1], F32, tag="srecip")
        nc.vector.reciprocal(srecip, os_ps[:, 128:129])
        out_tmp = big.tile([128, 128], F32, tag="out_tmp")
        nc.vector.tensor_scalar(out_tmp, os_ps[:, 0:128], scalar1=srecip,
                                scalar2=None, op0=ALU.mult)
        out_sb = big.tile([128, 128], F32, tag="out_sb")
        nc.vector.tensor_add(out_sb, out_tmp, biasb_sb)

        # full [m=4r+h, f] result out; host extracts out[r, 32h:32h+32] =
        # out_sb[4r+h, 32h:32h+32] during unshard
        nc.sync.dma_start(out=outp[:], in_=out_sb[:])

    nc.compile()
    return nc


def host_prep(x, batch, seed_nodes, W_l, W_r, att, bias, n_cap=N_CAP):
    f32 = np.float32
    x = np.asarray(x, f32)
    batch = np.asarray(batch).astype(np.int32)
    seed_nodes = np.asarray(seed_nodes, f32)
    W_l = np.asarray(W_l, f32)
    W_r = np.asarray(W_r, f32)
    att = np.asarray(att, f32)
    bias = np.asarray(bias, f32)

    order = np.argsort(batch, kind="stable")
    x_sorted = x[order]
    counts = np.bincount(batch[order], minlength=B)
    offs = np.concatenate([[0], np.cumsum(counts)])

    seed_hr = seed_nodes @ W_r                       # [32,128]
    A = np.zeros((F, H), f32)
    for h in range(H):
        A[h * C:(h + 1) * C, h] = att[h]
    A8 = (0.8 * A).astype(NP_BF16)
    # A_big[:, 128r + 4r + h] = 0.8*A[:, h]; zero elsewhere. Per-seed matmuls
    # use the prefix slice [:, 128r : 128r+4(r+1)] so output base partition
    # stays 0 while only rows 4r:4r+4 receive nonzero contributions.
    A_big = np.zeros((F, RATIO * 128), np.float32)
    for r in range(RATIO):
        A_big[:, 128 * r + 4 * r:128 * r + 4 * r + 4] = 0.8 * A
    A_big = A_big.astype(NP_BF16)
    Q = 0.2 * np.einsum("rf,fh->hr", seed_hr, A)     # [4,32]
    SelQ = np.zeros((6, 128), f32)
    m = np.arange(128)
    SelQ[m % 4, m] = 0.25
    SelQ[4, m] = Q[m % 4, m // 4]
    SelQ[5, :] = 1.0

    shared = dict(
        W_l=np.ascontiguousarray(W_l.astype(NP_BF16)),
        srT=np.ascontiguousarray(seed_hr.T),
        A8=np.ascontiguousarray(A8),
        A_big=A_big,
        SelQ=np.ascontiguousarray(SelQ.astype(NP_BF16)),
        bias1=np.ascontiguousarray(bias[None, :]),
    )
    in_maps = []
    for b in range(B):
        n_b = int(counts[b])
        assert n_b <= n_cap, f"graph {b}: {n_b} nodes > N_CAP {n_cap}"
        xb = np.zeros((n_cap, F), f32)
        xb[:n_b] = x_sorted[offs[b]:offs[b + 1]]
        maskbias = np.zeros((2, n_cap), f32)
        maskbias[0, :] = 1.0            # multiplies SelQ row 4 (Q correction)
        maskbias[1, n_b:] = -50.0       # multiplies SelQ row 5 (ones)
        in_maps.append(dict(
            shared,
            xT=np.ascontiguousarray(xb.T.astype(NP_BF16)),
            maskbias=maskbias.astype(NP_BF16),
        ))
    return in_maps


def kernel(x, batch, seed_nodes, W_l, W_r, att, bias):
    if "nc" not in _CACHE:
        _CACHE["nc"] = build_bass()
    nc = _CACHE["nc"]
    in_maps = host_prep(x, batch, seed_nodes, W_l, W_r, att, bias)
    res = run_bass_kernel_spmd(nc, in_maps, core_ids=list(range(B)))
    out = np.concatenate([unshard_core(np.asarray(res.results[i]["out"]))
                          for i in range(B)], axis=0)
    new_batch = np.repeat(np.arange(B, dtype=np.int32), RATIO)
    return out, new_batch


def unshard_core(out128):
    out = np.empty((RATIO, F), np.float32)
    for h in range(H):
        out[:, 32 * h:32 * (h + 1)] = out128[h::4, 32 * h:32 * (h + 1)]
    return out


# revision 18
# speedup vs baseline: 1.4516x; 1.4516x over previous
"""Trainium2 Bass kernel for nn_BipartitePooling (GATv2 bipartite pooling).

Sharding: one graph per NeuronCore (8 graphs, 8 cores, no collectives).
Per core, for its graph's nodes (padded to N_CAP columns):

  h_srcT[f,n] = W_l^T @ xT                      (PE)
  logit[4r+h, n] = A^T lrelu(h_srcT + srT[:,r]) (split as 0.8*relu + 0.2*linear:
      relu stream on ACT/DVE/GPSIMD, channel-reduce + linear correction + pad
      mask as PE matmuls accumulating in PSUM; softmax rows (r,h) are exactly
      the 128 partitions)
  p = exp(logit)                                (ACT; max-shift skipped,
                                                 logits are O(+-10))
  O[m,f], s[m] = p @ [h_src | 1]                (PE, via DMA-transposed p)
  out = O / s + bias                            (DVE)

n is processed in two column-halves so half 0's exp/transpose tail hides
under half 1's relu/matmul loop. Seed-side prep (srT = seed@W_r, block-diag
att groups, Q correction, pad masks) is tiny weight preprocessing on host.
"""
import os
import sys
from contextlib import ExitStack

import numpy as np

for _p in ("/root/.axon_site", "/root/.axon_site/_ro/trn_rl_repo",
           "/root/.axon_site/_ro/pypackages", "/opt/trn_rl_repo"):
    if os.path.isdir(_p) and _p not in sys.path:
        sys.path.append(_p)

import concourse.bass as bass
import concourse.bacc as bacc
import concourse.tile as tile
import concourse.mybir as mybir
from concourse.bass_utils import run_bass_kernel_spmd

F32 = mybir.dt.float32
BF16 = mybir.dt.bfloat16
AF = mybir.ActivationFunctionType
ALU = mybir.AluOpType
NP_BF16 = mybir.dt.np(BF16)

B, F, RATIO, H, C = 8, 128, 32, 4, 32
N_CAP = 2304
HALVES = [(0, 1280), (1280, 1024)]      # column ranges per pipeline half

# relu-stream engine per seed index r: ACT / DVE split (plus optional GPSIMD)
ENGINE_OF_R = ["A" if r % 3 == 0 else "V" for r in range(RATIO)]
# traversal order interleaves the four 32-row PSUM column-groups (r//8) so
# consecutive PE matmuls target different col_grps of the array
R_ORDER = [8 * (i % 4) + i // 4 for i in range(RATIO)]

_CACHE = {}


def _chunks_of(off0, size):
    out = []
    o = 0
    while o < size:
        sz = min(512, size - o)
        out.append((off0 + o, sz))
        o += sz
    return out


def build_bass(n_cap=N_CAP):
    nc = bacc.Bacc("TRN2", target_bir_lowering=False, debug=False)
    xT = nc.declare_dram_parameter("xT", [128, n_cap], BF16, isOutput=False)
    wl = nc.declare_dram_parameter("W_l", [128, 128], BF16, isOutput=False)
    srT = nc.declare_dram_parameter("srT", [128, 32], F32, isOutput=False)
    abig = nc.declare_dram_parameter("A_big", [128, RATIO * 32], BF16,
                                     isOutput=False)
    selq = nc.declare_dram_parameter("SelQ", [6, 128], BF16, isOutput=False)
    mask = nc.declare_dram_parameter("maskones", [2, n_cap], BF16, isOutput=False)
    bias1 = nc.declare_dram_parameter("bias1", [1, 128], F32, isOutput=False)
    outp = nc.declare_dram_parameter("out", [128, 128], F32, isOutput=True)

    nt = n_cap // 128
    halves = HALVES if n_cap == N_CAP else [(0, n_cap)]

    with ExitStack() as ctx:
        tc = ctx.enter_context(tile.TileContext(nc))
        consts = ctx.enter_context(tc.tile_pool(name="consts", bufs=1))
        big = ctx.enter_context(tc.tile_pool(name="big", bufs=1))
        rpool = ctx.enter_context(tc.tile_pool(name="rpool", bufs=6))
        ps_tmp = ctx.enter_context(tc.tile_pool(name="ps_tmp", bufs=2, space="PSUM"))
        ps_lg = ctx.enter_context(tc.tile_pool(name="ps_lg", bufs=1, space="PSUM"))
        ps_os = ctx.enter_context(tc.tile_pool(name="ps_os", bufs=1, space="PSUM"))

        # ---- inputs in, spread across the three DMA issue queues ----
        xT_sb = big.tile([128, n_cap], BF16, tag="xT")
        nc.sync.dma_start(out=xT_sb, in_=xT[:])
        mask_sb = consts.tile([2, n_cap], BF16, tag="mask")
        nc.sync.dma_start(out=mask_sb, in_=mask[:])
        wl_sb = consts.tile([128, 128], BF16, tag="wl")
        nc.scalar.dma_start(out=wl_sb, in_=wl[:])
        abig_sb = consts.tile([128, RATIO * 32], BF16, tag="abig")
        nc.scalar.dma_start(out=abig_sb, in_=abig[:])
        selq_sb = consts.tile([6, 128], BF16, tag="selq")
        nc.gpsimd.dma_start(out=selq_sb, in_=selq[:])
        srT_sb = consts.tile([128, 32], F32, tag="srt")
        nc.gpsimd.dma_start(out=srT_sb, in_=srT[:])
        biasb_sb = consts.tile([128, 128], F32, tag="biasb")
        b1 = bias1[0:1, :]
        nc.gpsimd.dma_start(
            out=biasb_sb,
            in_=bass.AP(tensor=b1.tensor, offset=b1.offset, ap=[[0, 128], [1, 128]]),
        )

        # ---- h_srcT = W_l^T @ xT ----
        hsrcT_sb = big.tile([128, n_cap], BF16, tag="hsrcT")
        for off, sz in _chunks_of(0, n_cap):
            pt = ps_tmp.tile([128, 512], F32, tag="tmp")
            nc.tensor.matmul(pt[:, :sz], lhsT=wl_sb, rhs=xT_sb[:, off:off + sz],
                             start=True, stop=True)
            nc.vector.tensor_copy(hsrcT_sb[:, off:off + sz], pt[:, :sz])

        # ---- h_src tiles [n,f] (+ ones column per tile for the s-sum),
        #      4 node-tiles batched per PSUM bank ----
        hsrc_sb = big.tile([128, nt * 129], BF16, tag="hsrc")
        nc.vector.memset(hsrc_sb[:, 128::129], 1.0)
        for t0 in range(0, nt, 4):
            tn = min(4, nt - t0)
            pt = ps_tmp.tile([128, 512], F32, tag="tmp")
            for j in range(tn):
                t = t0 + j
                nc.tensor.matmul(pt[:, 128 * j:128 * (j + 1)],
                                 lhsT=xT_sb[:, 128 * t:128 * (t + 1)],
                                 rhs=wl_sb, start=True, stop=True)
            src = pt[:, :128 * tn].rearrange("p (t f) -> p t f", t=tn)
            dst = hsrc_sb[:, 129 * t0:129 * (t0 + tn)].rearrange(
                "p (t f) -> p t f", f=129)[:, :, 0:128]
            nc.vector.tensor_copy(dst, src)

        # ---- linear-term rhs: rows 0-3 = A8^T h_srcT, rows 4-5 = ones/mask ----
        rhsP_sb = big.tile([6, n_cap], BF16, tag="rhsP")
        nc.sync.dma_start(out=rhsP_sb[4:6, :], in_=mask_sb[:])
        for off, sz in _chunks_of(0, n_cap):
            pt = ps_tmp.tile([128, 512], F32, tag="tmp")
            nc.tensor.matmul(pt[0:4, :sz], lhsT=abig_sb[:, 0:4],
                             rhs=hsrcT_sb[:, off:off + sz], start=True, stop=True)
            nc.vector.tensor_copy(rhsP_sb[0:4, off:off + sz], pt[0:4, :sz])

        # ---- per-half: logit chunks, relu stream, exp, transpose ----
        p_sb = big.tile([128, n_cap], BF16, tag="p")
        pT_sb = big.tile([128, nt * 128], BF16, tag="pT")
        logit_sb = big.tile([128, n_cap], F32, tag="logit")
        trans_engines = [nc.sync, nc.scalar]

        for hi, (h_off, h_sz) in enumerate(halves):
            chunks = _chunks_of(h_off, h_sz)
            lg = []
            for ci, (off, sz) in enumerate(chunks):
                g = ps_lg.tile([128, sz], F32, tag=f"lg{hi}_{ci}")
                nc.tensor.matmul(g[:, :], lhsT=selq_sb,
                                 rhs=rhsP_sb[:, off:off + sz],
                                 start=True, stop=False, skip_group_check=True)
                lg.append(g)

            for r in R_ORDER:
                R = rpool.tile([128, h_sz], BF16, tag=f"R{hi}")
                bcol = srT_sb[:, r:r + 1]
                src = hsrcT_sb[:, h_off:h_off + h_sz]
                eng = ENGINE_OF_R[r]
                if eng == "A":
                    nc.scalar.activation(R, src, AF.Relu, bias=bcol, scale=1.0)
                elif eng == "G":
                    nc.gpsimd.tensor_scalar(R, src, scalar1=bcol, scalar2=0.0,
                                            op0=ALU.add, op1=ALU.max)
                else:
                    nc.vector.tensor_scalar(R, src, scalar1=bcol, scalar2=0.0,
                                            op0=ALU.add, op1=ALU.max)
                g32 = 32 * (r // 8)
                for ci, (off, sz) in enumerate(chunks):
                    nc.tensor.matmul(lg[ci][g32:g32 + 32, :],
                                     lhsT=abig_sb[:, 32 * r:32 * r + 32],
                                     rhs=R[:, off - h_off:off - h_off + sz],
                                     start=False, stop=(r % 8 == 7),
                                     tile_position=(0, g32),
                                     skip_group_check=True)

            for ci, (off, sz) in enumerate(chunks):
                nc.vector.tensor_copy(logit_sb[:, off:off + sz], lg[ci][:, :])
            nc.scalar.activation(p_sb[:, h_off:h_off + h_sz],
                                 logit_sb[:, h_off:h_off + h_sz], AF.Exp)
            for i, t in enumerate(range(h_off // 128, (h_off + h_sz) // 128)):
                trans_engines[i % 2].dma_start_transpose(
                    pT_sb[:, 128 * t:128 * (t + 1)],
                    p_sb[:, 128 * t:128 * (t + 1)])

        # ---- O[m,f] and s[m] in one accumulation over all node tiles ----
        os_ps = ps_os.tile([128, 129], F32, tag="os")
        for t in range(nt):
            nc.tensor.matmul(os_ps[:, :], lhsT=pT_sb[:, 128 * t:128 * (t + 1)],
                             rhs=hsrc_sb[:, 129 * t:129 * (t + 1)],
                             start=(t == 0), stop=(t == nt - 1))

        srecip = big.tile([128, 1], F32, tag="srecip")
        nc.vector.reciprocal(srecip, os_ps[:, 128:129])
        out_tmp = big.tile([128, 128], F32, tag="out_tmp")
        nc.vector.tensor_scalar(out_tmp, os_ps[:, 0:128], scalar1=srecip,
                                scalar2=None, op0=ALU.mult)
        out_sb = big.tile([128, 128], F32, tag="out_sb")
        nc.vector.tensor_add(out_sb, out_tmp, biasb_sb)

        # full [m=4r+h, f] result out; host extracts out[r, 32h:32h+32] =
        # out_sb[4r+h, 32h:32h+32] during unshard
        nc.sync.dma_start(out=outp[:], in_=out_sb[:])

    nc.compile()
    return nc


def host_prep(x, batch, seed_nodes, W_l, W_r, att, bias, n_cap=N_CAP):
    f32 = np.float32
    x = np.asarray(x, f32)
    batch = np.asarray(batch).astype(np.int32)
    seed_nodes = np.asarray(seed_nodes, f32)
    W_l = np.asarray(W_l, f32)
    W_r = np.asarray(W_r, f32)
    att = np.asarray(att, f32)
    bias = np.asarray(bias, f32)

    order = np.argsort(batch, kind="stable")
    x_sorted = x[order]
    counts = np.bincount(batch[order], minlength=B)
    offs = np.concatenate([[0], np.cumsum(counts)])

    seed_hr = seed_nodes @ W_r                       # [32,128]
    A = np.zeros((F, H), f32)
    for h in range(H):
        A[h * C:(h + 1) * C, h] = att[h]
    # A_big[:, 32r + 4(r%8) + h] = 0.8*A[:, h]; the per-seed matmul uses the
    # 32-col slice [:, 32r:32r+32] writing PSUM rows [32*(r//8), +32) so
    # row m = 4r+h while output base partitions stay 32-aligned.
    A_big = np.zeros((F, RATIO * 32), f32)
    for r in range(RATIO):
        A_big[:, 32 * r + 4 * (r % 8):32 * r + 4 * (r % 8) + 4] = 0.8 * A
    A_big = A_big.astype(NP_BF16)
    Q = 0.2 * np.einsum("rf,fh->hr", seed_hr, A)     # [4,32]
    SelQ = np.zeros((6, 128), f32)
    m = np.arange(128)
    SelQ[m % 4, m] = 0.25
    SelQ[4, m] = Q[m % 4, m // 4]
    SelQ[5, :] = 1.0

    shared = dict(
        W_l=np.ascontiguousarray(W_l.astype(NP_BF16)),
        srT=np.ascontiguousarray(seed_hr.T),
        A_big=A_big,
        SelQ=np.ascontiguousarray(SelQ.astype(NP_BF16)),
        bias1=np.ascontiguousarray(bias[None, :]),
    )
    in_maps = []
    for b in range(B):
        n_b = int(counts[b])
        assert n_b <= n_cap, f"graph {b}: {n_b} nodes > N_CAP {n_cap}"
        xb = np.zeros((n_cap, F), f32)
        xb[:n_b] = x_sorted[offs[b]:offs[b + 1]]
        maskones = np.zeros((2, n_cap), f32)
        maskones[0, :] = 1.0            # multiplies SelQ row 4 (Q correction)
        maskones[1, n_b:] = -50.0       # multiplies SelQ row 5 (ones)
        in_maps.append(dict(
            shared,
            xT=np.ascontiguousarray(xb.T.astype(NP_BF16)),
            maskones=maskones.astype(NP_BF16),
        ))
    return in_maps


def kernel(x, batch, seed_nodes, W_l, W_r, att, bias):
    if "nc" not in _CACHE:
        _CACHE["nc"] = build_bass()
    nc = _CACHE["nc"]
    in_maps = host_prep(x, batch, seed_nodes, W_l, W_r, att, bias)
    res = run_bass_kernel_spmd(nc, in_maps, core_ids=list(range(B)))
    out = np.concatenate([unshard_core(np.asarray(res.results[i]["out"]))
                          for i in range(B)], axis=0)
    new_batch = np.repeat(np.arange(B, dtype=np.int32), RATIO)
    return out, new_batch


def unshard_core(out128):
    out = np.empty((RATIO, F), np.float32)
    for h in range(H):
        out[:, 32 * h:32 * (h + 1)] = out128[h::4, 32 * h:32 * (h + 1)]
    return out
